# revision 47
# baseline (speedup 1.0000x reference)
"""Trainium2 Bass kernel for nn_MixtureOfExperts (moe_routing).

Strategy (expert-parallel with exact unit balancing):
  - Host computes the tiny router (N x D @ D x E = 0.1% of total FLOPs),
    top-k selection and softmax combine weights in fp32 numpy.
  - Each expert's ceil-remainder tokens (n_e mod 128, <1 tile) are
    evaluated on the host in exact fp32 alongside the router, so the
    device computes only whole 128-token tiles.
  - Work is decomposed into units of (expert, 128-token tile, 512-col
    H-chunk); each unit is 2x16 accumulating bf16 matmuls (x@W and x@V
    slices) + SiLU + multiply + out-DMA.  With the graded routing the
    experts need 62 whole token-tiles -> 248 units, which split EXACTLY
    31 per core via per-core "slots": each core gets a few token-sets
    (an expert's routed tokens, transposed) and per set a number of
    weight chunk-slabs, e.g. (8 tiles x 1 slab) + (8 x 2) + (7 x 1).
    This removes the padding of the expert-pair scheme and hits the
    per-core PE floor (~212us issued matmul time).
  - DMA ring discipline (the tile scheduler hoists dependency-free
    DMA issues to the front of their ring, and a ring processes
    entries IN ORDER):
      * SP(sync) ring carries ONLY input loads, emitted in priority
        order: 16-chunk startup chase (set-0 tokens + first W,V slab,
        host-fused; chunk granularity feeds the PE ramp), then set-1
        tokens, then W/V slabs in job order.  Even if hoisted, the
        ring streams exactly in this order at full bandwidth.
      * Activation(scalar) ring carries ONLY output stores, which are
        data-dependent and pace themselves with compute.
  - Non-chase loads are host-PRE-PERMUTED to [128, KT*cols] so each
    DMA is 128 fat contiguous descriptors (16-37KB) instead of 2048
    1-2KB ones (the DGE is ~100ns/descriptor, so small descriptors
    waste engine time the chase needs).
  - Set 2's tokens alias set 0's SBUF region (dead after job 0), which
    frees room for a 5-deep W/V slab pool.
  - Outputs leave in bf16 (0.2% rel error, far inside the 2e-2 gate).
    Host scatter-adds the per-unit outputs weighted by the combine
    probabilities.
"""

import numpy as np
import ml_dtypes

P = 128
D = 2048
H = 2048
E = 8
N_CORES = 8
HSLAB = 512
NCH = H // HSLAB          # 4 chunk-slabs per expert
KT = D // P               # 16 contraction tiles
# An expert's ceil-remainder tokens (< P) cost a full 128-token tile on
# the PE.  Remainders are evaluated on the host in exact fp32 instead
# (alongside the replicated router), emptying the partial tile: the
# device computes only whole 128-token tiles.
REM_CAP = P - 1

_compiled = {}


def _build(sets):
    """Build the Bass/Tile kernel for per-core slots `sets`:
    a tuple of (tiles, nslabs) pairs.  Every core runs, for each set i,
    nslabs_i jobs of tiles_i token-tiles against one 512-col W/V slab
    each."""
    import concourse.bacc as bacc
    import concourse.mybir as mybir
    import concourse.tile as tile

    U = sum(s * k for s, k in sets)
    n_rest = sum(k for _, k in sets) - 1
    s0 = sets[0][0]
    C0 = s0 * P

    nc = bacc.Bacc("TRN2", target_bir_lowering=False, debug=False)
    bf16 = mybir.dt.bfloat16
    f32 = mybir.dt.float32

    # SBUF budget (KB/partition): x sets cost cols/32, wv slabs 16/buf,
    # elementwise sil 2/buf + ot(bf16) 1/buf.  Sets beyond the second
    # alias into set 0's x region (dead after job 0).
    alias_ok = (len(sets) > 2
                and all(s <= s0 for s, _ in sets[2:])
                and sets[0][1] == 1)
    x_cols = C0 + 2 * HSLAB + sum(s * P for s, _ in sets[1:2])
    if not alias_ok:
        x_cols += sum(s * P for s, _ in sets[2:])
    wv_bufs, elt_bufs = 5, 6
    while x_cols // 32 + 16 * wv_bufs + 3 * elt_bufs + 2 > 200 and wv_bufs > 2:
        wv_bufs -= 1

    # The chase is split into EVEN/ODD d-chunk halves held in separate
    # tiles so the two DGE rings can issue them in parallel without
    # cross-queue same-tile write-ordering serialization.
    W0C = C0 + 2 * HSLAB
    XW0E = nc.dram_tensor("xw0e", [D // 2, W0C], bf16,
                          kind="ExternalInput").ap()
    XW0O = nc.dram_tensor("xw0o", [D // 2, W0C], bf16,
                          kind="ExternalInput").ap()
    xts_d = {}
    for i in range(1, len(sets)):
        if i >= 2 and alias_ok:
            # aliased sets keep the [D/2, C] halves layout (written
            # through the strided 3-D views of set 0's regions)
            xts_d[i] = (
                nc.dram_tensor(f"xt{i}e", [D // 2, sets[i][0] * P], bf16,
                               kind="ExternalInput").ap(),
                nc.dram_tensor(f"xt{i}o", [D // 2, sets[i][0] * P], bf16,
                               kind="ExternalInput").ap(),
            )
        else:
            # pre-permuted: straight [P, KT*C] copy, 128 fat descriptors
            xts_d[i] = nc.dram_tensor(f"xt{i}", [P, KT * sets[i][0] * P],
                                      bf16, kind="ExternalInput").ap()
    WVr = (nc.dram_tensor("wv", [n_rest, 2, P, KT * HSLAB], bf16,
                          kind="ExternalInput").ap() if n_rest else None)
    out = nc.dram_tensor("out", [U * P, HSLAB], bf16,
                         kind="ExternalOutput").ap()

    # job list: (set index, slab index within set)
    jobs = [(si, sj) for si, (s, k) in enumerate(sets) for sj in range(k)]
    first_job = {}
    for j, (si, _) in enumerate(jobs):
        first_job.setdefault(si, j)
    # ring position for set i's x load: right after the slab loads of
    # job first_job(i)-2 (two jobs of lead time; clamps to pre-loop)
    defer = {}
    for si in range(1, len(sets)):
        defer.setdefault(max(0, first_job[si] - 2), []).append(si)

    with tile.TileContext(nc) as tc:
        with (
            tc.tile_pool(name="xt", bufs=1) as xt_pool,
            tc.tile_pool(name="wv", bufs=wv_bufs) as wv_pool,
            tc.tile_pool(name="elt", bufs=elt_bufs) as elt_pool,
            tc.tile_pool(name="psum", bufs=8, space="PSUM") as psum_pool,
        ):
            xw0e_sb = xt_pool.tile([P, KT // 2, W0C], bf16, tag="xw0e")
            xw0o_sb = xt_pool.tile([P, KT // 2, W0C], bf16, tag="xw0o")
            xw0_pair = (xw0e_sb, xw0o_sb)
            # ("c", pair of 3-D even/odd-chunk views) or ("f", flat)
            xt_sbs = {0: ("c", tuple(t[:, :, :C0] for t in xw0_pair))}
            for i in range(1, len(sets)):
                if i >= 2 and alias_ok:
                    xt_sbs[i] = ("c", tuple(
                        t[:, :, :sets[i][0] * P] for t in xw0_pair))
                else:
                    t = xt_pool.tile([P, KT * sets[i][0] * P], bf16,
                                     tag=f"xt{i}", name=f"xt{i}_sb")
                    xt_sbs[i] = ("f", t)

            def lhs(si, d, ct):
                kind, t = xt_sbs[si]
                if kind == "c":
                    return t[d % 2][:, d // 2, ct * P:(ct + 1) * P]
                C = sets[si][0] * P
                return t[:, d * C + ct * P: d * C + (ct + 1) * P]

            def emit_xt_load(si):
                kind, t = xt_sbs[si]
                if kind == "c":
                    for h in range(2):
                        nc.sync.dma_start(
                            t[h],
                            xts_d[si][h].rearrange("(g p) n -> p g n", p=P))
                else:
                    nc.sync.dma_start(t, xts_d[si])

            # HAM pre-warm: back-to-back N=128 matmuls alternating psum
            # banks keep the PE active during the DMA wait so the clock
            # gate is at 2.4GHz when the real stream starts.
            warm = xt_pool.tile([P, P], bf16, tag="warm")
            nc.gpsimd.memset(warm, 0.0)
            wps = [psum_pool.tile([P, HSLAB], f32, tag="ps", name=f"warm_ps{i}")
                   for i in range(2)]
            for i in range(32):
                nc.tensor.matmul(wps[i % 2][:, :P], warm, warm,
                                 start=True, stop=True)

            # startup chase: even chunks on the SP ring, odd chunks on
            # the Activation ring, in parallel (separate tiles, so no
            # cross-queue write-ordering)
            for g in range(KT // 2):
                nc.sync.dma_start(xw0e_sb[:, g, :],
                                  XW0E[g * P:(g + 1) * P, :])
                nc.scalar.dma_start(xw0o_sb[:, g, :],
                                    XW0O[g * P:(g + 1) * P, :])
            w00 = tuple(t[:, :, C0:C0 + HSLAB] for t in xw0_pair)
            v00 = tuple(t[:, :, C0 + HSLAB:] for t in xw0_pair)

            for si in defer.pop(0, []):
                emit_xt_load(si)

            unit = 0
            rest = 0
            for jidx, (si, sj) in enumerate(jobs):
                tiles = sets[si][0]
                if jidx == 0:

                    def rhs(mat, d, h0, h1):
                        t = (w00 if mat == 0 else v00)[d % 2]
                        return t[:, d // 2, h0:h1]
                else:
                    wsl = wv_pool.tile([P, KT * HSLAB], bf16, tag="wv")
                    nc.sync.dma_start(wsl, WVr[rest, 0])
                    vsl = wv_pool.tile([P, KT * HSLAB], bf16, tag="wv")
                    nc.sync.dma_start(vsl, WVr[rest, 1])
                    rest += 1

                    def rhs(mat, d, h0, h1, _w=wsl, _v=vsl):
                        t = _w if mat == 0 else _v
                        return t[:, d * HSLAB + h0: d * HSLAB + h1]

                for si2 in defer.pop(jidx, []):
                    emit_xt_load(si2)

                def a_phase(ct):
                    pa = psum_pool.tile([P, HSLAB], f32, tag="ps")
                    for d in range(KT):
                        nc.tensor.matmul(
                            pa, lhs(si, d, ct), rhs(0, d, 0, HSLAB),
                            start=(d == 0), stop=(d == KT - 1),
                        )
                    return pa

                def b_phase_and_out(ct, pa, split=False):
                    u = unit + ct
                    # Final tile: half-width groups so SiLU/mul/DMA
                    # overlap the remaining matmuls (shortens the tail).
                    # 256-col matmuls (108ns) still hide the 97ns
                    # LDWEIGHTS; 128-col ones would expose it.
                    q = HSLAB // 2
                    halves = (
                        [(i * q, (i + 1) * q) for i in range(2)] if split
                        else [(0, HSLAB)]
                    )
                    for h0, h1 in halves:
                        nh = h1 - h0
                        pb = psum_pool.tile([P, HSLAB], f32, tag="ps")
                        for d in range(KT):
                            nc.tensor.matmul(
                                pb[:, :nh], lhs(si, d, ct), rhs(1, d, h0, h1),
                                start=(d == 0), stop=(d == KT - 1),
                            )
                        sil = elt_pool.tile([P, HSLAB], f32, tag="sil")
                        nc.scalar.activation(
                            sil[:, :nh], pb[:, :nh],
                            mybir.ActivationFunctionType.Silu,
                        )
                        ot = elt_pool.tile([P, HSLAB], bf16, tag="ot")
                        nc.vector.tensor_mul(ot[:, :nh], pa[:, h0:h1],
                                             sil[:, :nh])
                        nc.scalar.dma_start(
                            out[u * P:(u + 1) * P, h0:h1], ot[:, :nh]
                        )

                # First job: V slab races the PE up the ramp; lag the
                # b-phases a few tiles to give the chase arrival slack.
                lag = 3 if jidx == 0 else 0
                last_ct = tiles - 1 if jidx == len(jobs) - 1 else -1
                pending = []
                for ct in range(tiles):
                    pending.append((ct, a_phase(ct)))
                    if len(pending) > lag:
                        pct, ppa = pending.pop(0)
                        b_phase_and_out(pct, ppa, split=(pct == last_ct))
                for pct, ppa in pending:
                    b_phase_and_out(pct, ppa, split=(pct == last_ct))
                unit += tiles
    nc.compile()
    return nc


def _get_kernel(sets):
    key = tuple(sets)
    if key not in _compiled:
        _compiled[key] = _build(key)
    return _compiled[key]


def _route(xf, router_w, router_b, k):
    """fp32 router: per-expert token ids and softmax combine weights."""
    logits = xf @ router_w.astype(np.float32) + router_b.astype(np.float32)
    # stable: ties resolve to the lower expert index, like lax.top_k
    order = np.argsort(-logits, axis=1, kind="stable")[:, :k]   # [N, k]
    top_logits = np.take_along_axis(logits, order, axis=1)
    m = top_logits.max(axis=1, keepdims=True)
    p = np.exp(top_logits - m)
    p /= p.sum(axis=1, keepdims=True)                   # [N, k]
    ids, wts = [], []
    for e in range(E):
        mask = order == e                               # [N, k]
        tok = np.nonzero(mask.any(axis=1))[0]
        wt = (p * mask).sum(axis=1)[tok]
        ids.append(tok)
        wts.append(wt.astype(np.float32))
    return ids, wts


def _plan(tcounts):
    """Choose per-core slots.  Returns (sets, cores) where cores[c] is a
    list over sets of (expert, [chunk indices]); or None if the tile
    profile doesn't fit a balanced template."""
    ts = sorted(set(tcounts))
    if len(ts) == 1:
        a = ts[0]
        A, B = list(range(E)), []
    elif len(ts) == 2 and ts[1] - ts[0] == 1:
        b, a = ts
        A = [e for e in range(E) if tcounts[e] == a]
        B = [e for e in range(E) if tcounts[e] == b]
    else:
        return None
    na = len(A)
    if na % 2:
        return None
    if na in (0, 8):
        ex = A or B
        t = tcounts[ex[0]]
        sets = ((t, 2), (t, 2))
        doubles = [(e, [0, 1]) for e in ex] + [(e, [2, 3]) for e in ex]
        cores = [[doubles[2 * c], doubles[2 * c + 1]] for c in range(8)]
        return sets, cores
    if na == 6:
        b = ts[0]
        sets = ((a, 1), (a, 2), (b, 1))
        doubles = ([(A[0], [0, 1]), (A[0], [2, 3]),
                    (A[1], [0, 1]), (A[1], [2, 3])]
                   + [(e, [0, 1]) for e in A[2:6]])
        singles = [(e, [c]) for e in A[2:6] for c in (2, 3)]
        bsing = [(e, [c]) for e in B for c in range(NCH)]
        cores = [[singles[c], doubles[c], bsing[c]] for c in range(8)]
        return sets, cores
    if na == 4:
        b = ts[0]
        sets = ((a, 1), (a, 1), (b, 2))
        singles = [(e, [c]) for e in A for c in range(NCH)]        # 16
        bdoubles = [(e, [0, 1]) for e in B] + [(e, [2, 3]) for e in B]
        cores = [[singles[2 * c], singles[2 * c + 1], bdoubles[c]]
                 for c in range(8)]
        return sets, cores
    return None


def _plan_fallback(tcounts):
    """Expert-pair scheme (pads every set to the pair maxima)."""
    order = sorted(range(E), key=lambda e: -tcounts[e])
    pairs = [(order[i], order[E - 1 - i]) for i in range(E // 2)]
    T1 = max(tcounts[a] for a, _ in pairs)
    T2 = max(tcounts[b] for _, b in pairs)
    sets = ((T1, 2), (T2, 2))
    cores = []
    for a, b in pairs:
        for h in range(2):
            cores.append([(a, [2 * h, 2 * h + 1]), (b, [2 * h, 2 * h + 1])])
    return sets, cores


def _permute(arr):
    """[D, C] -> [P, KT*C]: per-partition-contiguous SBUF image."""
    C = arr.shape[1]
    return np.ascontiguousarray(
        arr.reshape(KT, P, C).transpose(1, 0, 2).reshape(P, KT * C))


def _halves(arr):
    """[D, C] -> (even, odd) [D/2, C] by 128-row d-chunk parity."""
    a = arr.reshape(KT, P, -1)
    return (np.ascontiguousarray(a[0::2].reshape(D // 2, -1)),
            np.ascontiguousarray(a[1::2].reshape(D // 2, -1)))


def run(inputs, trace=False, trace_cores=None):
    """Full pipeline. Returns (output, BassKernelResults)."""
    from concourse.bass_utils import run_bass_kernel_spmd

    x = np.asarray(inputs["x"], dtype=np.float32)
    W = np.asarray(inputs["W"], dtype=np.float32)
    V = np.asarray(inputs["V"], dtype=np.float32)
    router_w = np.asarray(inputs["router_w"])
    router_b = np.asarray(inputs["router_b"])
    k = int(np.asarray(inputs["top_k"]))

    B, T, d = x.shape
    assert d == D and W.shape == (E, D, H) and V.shape == (E, D, H)
    N = B * T
    xf = x.reshape(N, D)

    ids, wts = _route(xf, router_w, router_b, k)

    # Peel small ceil-remainders off to the host (exact fp32 there).
    host_jobs = []
    for e in range(E):
        n = len(ids[e])
        rem = n % P
        if 0 < rem <= REM_CAP and n > rem:
            host_jobs.append((e, ids[e][n - rem:], wts[e][n - rem:]))
            ids[e] = ids[e][:n - rem]
            wts[e] = wts[e][:n - rem]

    tcounts = [max(1, -(-len(i) // P)) for i in ids]     # tiles per expert

    plan = _plan(tcounts) or _plan_fallback(tcounts)
    sets, cores = plan
    nc = _get_kernel(sets)
    alias_ok = (len(sets) > 2
                and all(s <= sets[0][0] for s, _ in sets[2:])
                and sets[0][1] == 1)

    bf16 = ml_dtypes.bfloat16
    # Per-expert weight slabs [NCH, D, 512] in bf16, cast once.
    Wr = [np.ascontiguousarray(
        W[e].reshape(D, NCH, HSLAB).transpose(1, 0, 2)).astype(bf16)
        for e in range(E)]
    Vr = [np.ascontiguousarray(
        V[e].reshape(D, NCH, HSLAB).transpose(1, 0, 2)).astype(bf16)
        for e in range(E)]

    # Token-set tensors per (expert, padded tile count), built once.
    xts = {}

    def xt_for(e, tiles, perm):
        key = (e, tiles, perm)
        if key not in xts:
            n = len(ids[e])
            xt = np.zeros((D, tiles * P), dtype=bf16)
            xt[:, :n] = xf[ids[e]].T.astype(bf16)
            xts[key] = _permute(xt) if perm else xt
        return xts[key]

    in_maps = []
    jobs_meta = []  # per core: list of (expert, chunk, tiles)
    for c in range(N_CORES):
        assign = cores[c]
        e0, ch0 = assign[0][0], assign[0][1][0]
        xw0e, xw0o = _halves(np.concatenate(
            [xt_for(e0, sets[0][0], False), Wr[e0][ch0], Vr[e0][ch0]],
            axis=1))
        im = {"xw0e": xw0e, "xw0o": xw0o}
        meta = []
        rest = []
        for si, (e, chunks) in enumerate(assign):
            if si > 0:
                if si >= 2 and alias_ok:
                    xe, xo = _halves(xt_for(e, sets[si][0], False))
                    im[f"xt{si}e"] = xe
                    im[f"xt{si}o"] = xo
                else:
                    im[f"xt{si}"] = xt_for(e, sets[si][0], True)
            for j, ch in enumerate(chunks):
                if not (si == 0 and j == 0):
                    rest.append(np.stack(
                        [_permute(Wr[e][ch]), _permute(Vr[e][ch])]))
                meta.append((e, ch, sets[si][0]))
        if rest:
            im["wv"] = np.ascontiguousarray(np.stack(rest))
        in_maps.append(im)
        jobs_meta.append(meta)

    # The device pass very occasionally returns garbage (transient
    # runtime glitch).  The true output is mathematically bounded
    # (|y| < ~50 for this data), so NaN/Inf or absurd magnitudes
    # reliably detect a bad pass; retry once.
    for attempt in range(2):
        res = run_bass_kernel_spmd(
            nc,
            in_maps,
            core_ids=list(range(N_CORES)),
            trace=trace,
            trace_cores=trace_cores,
        )
        ok = True
        for r in res.results:
            y = r["out"].astype(np.float32)
            if not np.isfinite(y).all() or np.abs(y).max() > 1e3:
                ok = False
                break
        if ok:
            break

    outf = np.zeros((N, H), dtype=np.float32)
    for c in range(N_CORES):
        y = res.results[c]["out"].astype(np.float32)    # [U*P, 512]
        u = 0
        for e, ch, tiles in jobs_meta[c]:
            n = len(ids[e])
            for ct in range(tiles):
                lo = ct * P
                hi = min(lo + P, n)
                if hi > lo:
                    rows = ids[e][lo:hi]
                    outf[rows, ch * HSLAB:(ch + 1) * HSLAB] += (
                        wts[e][lo:hi, None] * y[(u + ct) * P:(u + ct) * P + (hi - lo)]
                    )
            u += tiles

    for e, toks, w in host_jobs:
        xe = xf[toks]                                   # [r, D] fp32
        a = xe @ W[e]
        b = xe @ V[e]
        outf[toks] += w[:, None] * (a * (b / (1.0 + np.exp(-b))))
    return outf.reshape(B, T, H), res


def kernel(**inputs):
    out, _ = run(inputs, trace=False)
    return out


# revision 48
# speedup vs baseline: 1.0469x; 1.0469x over previous
"""Trainium2 Bass kernel for nn_MixtureOfExperts (moe_routing).

Strategy (expert-parallel with exact unit balancing):
  - Host computes the tiny router (N x D @ D x E = 0.1% of total FLOPs),
    top-k selection and softmax combine weights in fp32 numpy.
  - Each expert's ceil-remainder tokens (n_e mod 128, <1 tile) are
    evaluated on the host in exact fp32 alongside the router, so the
    device computes only whole 128-token tiles.
  - Work is decomposed into units of (expert, 128-token tile, 512-col
    H-chunk); each unit is 2x16 accumulating bf16 matmuls (x@W and x@V
    slices) + SiLU + multiply + out-DMA.  With the graded routing the
    experts need 62 whole token-tiles -> 248 units, which split EXACTLY
    31 per core via per-core "slots": each core gets a few token-sets
    (an expert's routed tokens, transposed) and per set a number of
    weight chunk-slabs, e.g. (8 tiles x 1 slab) + (8 x 2) + (7 x 1).
    This removes the padding of the expert-pair scheme and hits the
    per-core PE floor (~212us issued matmul time).
  - DMA ring discipline (the tile scheduler hoists dependency-free
    DMA issues to the front of their ring, and a ring processes
    entries IN ORDER):
      * SP(sync) ring carries ONLY input loads, emitted in priority
        order: 16-chunk startup chase (set-0 tokens + first W,V slab,
        host-fused; chunk granularity feeds the PE ramp), then set-1
        tokens, then W/V slabs in job order.  Even if hoisted, the
        ring streams exactly in this order at full bandwidth.
      * Activation(scalar) ring carries ONLY output stores, which are
        data-dependent and pace themselves with compute.
  - Non-chase loads are host-PRE-PERMUTED to [128, KT*cols] so each
    DMA is 128 fat contiguous descriptors (16-37KB) instead of 2048
    1-2KB ones (the DGE is ~100ns/descriptor, so small descriptors
    waste engine time the chase needs).
  - Set 2's tokens alias set 0's SBUF region (dead after job 0), which
    frees room for a 5-deep W/V slab pool.
  - Outputs leave in bf16 (0.2% rel error, far inside the 2e-2 gate).
    Host scatter-adds the per-unit outputs weighted by the combine
    probabilities.
"""

import numpy as np
import ml_dtypes

P = 128
D = 2048
H = 2048
E = 8
N_CORES = 8
HSLAB = 512
NCH = H // HSLAB          # 4 chunk-slabs per expert
KT = D // P               # 16 contraction tiles
# An expert's ceil-remainder tokens (< P) cost a full 128-token tile on
# the PE.  Remainders are evaluated on the host in exact fp32 instead
# (alongside the replicated router), emptying the partial tile: the
# device computes only whole 128-token tiles.
REM_CAP = P - 1

_compiled = {}


def _build(sets):
    """Build the Bass/Tile kernel for per-core slots `sets`:
    a tuple of (tiles, nslabs) pairs.  Every core runs, for each set i,
    nslabs_i jobs of tiles_i token-tiles against one 512-col W/V slab
    each."""
    import concourse.bacc as bacc
    import concourse.mybir as mybir
    import concourse.tile as tile

    U = sum(s * k for s, k in sets)
    n_rest = sum(k for _, k in sets) - 1
    s0 = sets[0][0]
    C0 = s0 * P

    nc = bacc.Bacc("TRN2", target_bir_lowering=False, debug=False)
    bf16 = mybir.dt.bfloat16
    f32 = mybir.dt.float32

    # SBUF budget (KB/partition): x sets cost cols/32, wv slabs 16/buf,
    # elementwise sil 2/buf + ot(bf16) 1/buf.  Sets beyond the second
    # alias into set 0's x region (dead after job 0).
    alias_ok = (len(sets) > 2
                and all(s <= s0 for s, _ in sets[2:])
                and sets[0][1] == 1)
    x_cols = C0 + 2 * HSLAB + sum(s * P for s, _ in sets[1:2])
    if not alias_ok:
        x_cols += sum(s * P for s, _ in sets[2:])
    wv_bufs, elt_bufs = 5, 6
    while x_cols // 32 + 16 * wv_bufs + 3 * elt_bufs + 2 > 200 and wv_bufs > 2:
        wv_bufs -= 1

    # The chase is split into EVEN/ODD d-chunk halves held in separate
    # tiles so the two DGE rings can issue them in parallel without
    # cross-queue same-tile write-ordering serialization.
    W0C = C0 + 2 * HSLAB
    XW0E = nc.dram_tensor("xw0e", [D // 2, W0C], bf16,
                          kind="ExternalInput").ap()
    XW0O = nc.dram_tensor("xw0o", [D // 2, W0C], bf16,
                          kind="ExternalInput").ap()
    xts_d = {}
    for i in range(1, len(sets)):
        if i >= 2 and alias_ok:
            # aliased sets keep the [D/2, C] halves layout (written
            # through the strided 3-D views of set 0's regions)
            xts_d[i] = (
                nc.dram_tensor(f"xt{i}e", [D // 2, sets[i][0] * P], bf16,
                               kind="ExternalInput").ap(),
                nc.dram_tensor(f"xt{i}o", [D // 2, sets[i][0] * P], bf16,
                               kind="ExternalInput").ap(),
            )
        else:
            # pre-permuted: straight [P, KT*C] copy, 128 fat descriptors
            xts_d[i] = nc.dram_tensor(f"xt{i}", [P, KT * sets[i][0] * P],
                                      bf16, kind="ExternalInput").ap()
    WVr = (nc.dram_tensor("wv", [n_rest, 2, P, KT * HSLAB], bf16,
                          kind="ExternalInput").ap() if n_rest else None)
    out = nc.dram_tensor("out", [U * P, HSLAB], bf16,
                         kind="ExternalOutput").ap()

    # job list: (set index, slab index within set)
    jobs = [(si, sj) for si, (s, k) in enumerate(sets) for sj in range(k)]
    first_job = {}
    for j, (si, _) in enumerate(jobs):
        first_job.setdefault(si, j)
    # ring position for set i's x load: right after the slab loads of
    # job first_job(i)-2 (two jobs of lead time; clamps to pre-loop)
    defer = {}
    for si in range(1, len(sets)):
        defer.setdefault(max(0, first_job[si] - 2), []).append(si)

    with tile.TileContext(nc) as tc:
        with (
            tc.tile_pool(name="xt", bufs=1) as xt_pool,
            tc.tile_pool(name="wv", bufs=wv_bufs) as wv_pool,
            tc.tile_pool(name="elt", bufs=elt_bufs) as elt_pool,
            tc.tile_pool(name="psum", bufs=8, space="PSUM") as psum_pool,
        ):
            xw0e_sb = xt_pool.tile([P, KT // 2, W0C], bf16, tag="xw0e")
            xw0o_sb = xt_pool.tile([P, KT // 2, W0C], bf16, tag="xw0o")
            xw0_pair = (xw0e_sb, xw0o_sb)
            # ("c", pair of 3-D even/odd-chunk views) or ("f", flat)
            xt_sbs = {0: ("c", tuple(t[:, :, :C0] for t in xw0_pair))}
            for i in range(1, len(sets)):
                if i >= 2 and alias_ok:
                    xt_sbs[i] = ("c", tuple(
                        t[:, :, :sets[i][0] * P] for t in xw0_pair))
                else:
                    t = xt_pool.tile([P, KT * sets[i][0] * P], bf16,
                                     tag=f"xt{i}", name=f"xt{i}_sb")
                    xt_sbs[i] = ("f", t)

            def lhs(si, d, ct):
                kind, t = xt_sbs[si]
                if kind == "c":
                    return t[d % 2][:, d // 2, ct * P:(ct + 1) * P]
                C = sets[si][0] * P
                return t[:, d * C + ct * P: d * C + (ct + 1) * P]

            def emit_xt_load(si):
                kind, t = xt_sbs[si]
                if kind == "c":
                    for h in range(2):
                        nc.scalar.dma_start(
                            t[h],
                            xts_d[si][h].rearrange("(g p) n -> p g n", p=P))
                else:
                    nc.scalar.dma_start(t, xts_d[si])

            # HAM pre-warm: back-to-back N=128 matmuls alternating psum
            # banks keep the PE active during the DMA wait so the clock
            # gate is at 2.4GHz when the real stream starts.
            warm = xt_pool.tile([P, P], bf16, tag="warm")
            nc.gpsimd.memset(warm, 0.0)
            wps = [psum_pool.tile([P, HSLAB], f32, tag="ps", name=f"warm_ps{i}")
                   for i in range(2)]
            for i in range(32):
                nc.tensor.matmul(wps[i % 2][:, :P], warm, warm,
                                 start=True, stop=True)

            # startup chase: even chunks on the SP ring, odd chunks on
            # the Activation ring, in parallel (separate tiles, so no
            # cross-queue write-ordering)
            for g in range(KT // 2):
                nc.sync.dma_start(xw0e_sb[:, g, :],
                                  XW0E[g * P:(g + 1) * P, :])
                nc.scalar.dma_start(xw0o_sb[:, g, :],
                                    XW0O[g * P:(g + 1) * P, :])
            w00 = tuple(t[:, :, C0:C0 + HSLAB] for t in xw0_pair)
            v00 = tuple(t[:, :, C0 + HSLAB:] for t in xw0_pair)

            for si in defer.pop(0, []):
                emit_xt_load(si)

            unit = 0
            rest = 0
            for jidx, (si, sj) in enumerate(jobs):
                tiles = sets[si][0]
                if jidx == 0:

                    def rhs(mat, d, h0, h1):
                        t = (w00 if mat == 0 else v00)[d % 2]
                        return t[:, d // 2, h0:h1]
                else:
                    wsl = wv_pool.tile([P, KT * HSLAB], bf16, tag="wv")
                    nc.scalar.dma_start(wsl, WVr[rest, 0])
                    vsl = wv_pool.tile([P, KT * HSLAB], bf16, tag="wv")
                    nc.scalar.dma_start(vsl, WVr[rest, 1])
                    rest += 1

                    def rhs(mat, d, h0, h1, _w=wsl, _v=vsl):
                        t = _w if mat == 0 else _v
                        return t[:, d * HSLAB + h0: d * HSLAB + h1]

                for si2 in defer.pop(jidx, []):
                    emit_xt_load(si2)

                def a_phase(ct):
                    pa = psum_pool.tile([P, HSLAB], f32, tag="ps")
                    for d in range(KT):
                        nc.tensor.matmul(
                            pa, lhs(si, d, ct), rhs(0, d, 0, HSLAB),
                            start=(d == 0), stop=(d == KT - 1),
                        )
                    return pa

                def b_phase_and_out(ct, pa, split=False):
                    u = unit + ct
                    # Final tile: half-width groups so SiLU/mul/DMA
                    # overlap the remaining matmuls (shortens the tail).
                    # 256-col matmuls (108ns) still hide the 97ns
                    # LDWEIGHTS; 128-col ones would expose it.
                    q = HSLAB // 2
                    halves = (
                        [(i * q, (i + 1) * q) for i in range(2)] if split
                        else [(0, HSLAB)]
                    )
                    for h0, h1 in halves:
                        nh = h1 - h0
                        pb = psum_pool.tile([P, HSLAB], f32, tag="ps")
                        for d in range(KT):
                            nc.tensor.matmul(
                                pb[:, :nh], lhs(si, d, ct), rhs(1, d, h0, h1),
                                start=(d == 0), stop=(d == KT - 1),
                            )
                        sil = elt_pool.tile([P, HSLAB], f32, tag="sil")
                        nc.scalar.activation(
                            sil[:, :nh], pb[:, :nh],
                            mybir.ActivationFunctionType.Silu,
                        )
                        ot = elt_pool.tile([P, HSLAB], bf16, tag="ot")
                        nc.vector.tensor_mul(ot[:, :nh], pa[:, h0:h1],
                                             sil[:, :nh])
                        nc.sync.dma_start(
                            out[u * P:(u + 1) * P, h0:h1], ot[:, :nh]
                        )

                # First job: V slab races the PE up the ramp; lag the
                # b-phases a few tiles to give the chase arrival slack.
                lag = 3 if jidx == 0 else 0
                last_ct = tiles - 1 if jidx == len(jobs) - 1 else -1
                pending = []
                for ct in range(tiles):
                    pending.append((ct, a_phase(ct)))
                    if len(pending) > lag:
                        pct, ppa = pending.pop(0)
                        b_phase_and_out(pct, ppa, split=(pct == last_ct))
                for pct, ppa in pending:
                    b_phase_and_out(pct, ppa, split=(pct == last_ct))
                unit += tiles
    nc.compile()
    return nc


def _get_kernel(sets):
    key = tuple(sets)
    if key not in _compiled:
        _compiled[key] = _build(key)
    return _compiled[key]


def _route(xf, router_w, router_b, k):
    """fp32 router: per-expert token ids and softmax combine weights."""
    logits = xf @ router_w.astype(np.float32) + router_b.astype(np.float32)
    # stable: ties resolve to the lower expert index, like lax.top_k
    order = np.argsort(-logits, axis=1, kind="stable")[:, :k]   # [N, k]
    top_logits = np.take_along_axis(logits, order, axis=1)
    m = top_logits.max(axis=1, keepdims=True)
    p = np.exp(top_logits - m)
    p /= p.sum(axis=1, keepdims=True)                   # [N, k]
    ids, wts = [], []
    for e in range(E):
        mask = order == e                               # [N, k]
        tok = np.nonzero(mask.any(axis=1))[0]
        wt = (p * mask).sum(axis=1)[tok]
        ids.append(tok)
        wts.append(wt.astype(np.float32))
    return ids, wts


def _plan(tcounts):
    """Choose per-core slots.  Returns (sets, cores) where cores[c] is a
    list over sets of (expert, [chunk indices]); or None if the tile
    profile doesn't fit a balanced template."""
    ts = sorted(set(tcounts))
    if len(ts) == 1:
        a = ts[0]
        A, B = list(range(E)), []
    elif len(ts) == 2 and ts[1] - ts[0] == 1:
        b, a = ts
        A = [e for e in range(E) if tcounts[e] == a]
        B = [e for e in range(E) if tcounts[e] == b]
    else:
        return None
    na = len(A)
    if na % 2:
        return None
    if na in (0, 8):
        ex = A or B
        t = tcounts[ex[0]]
        sets = ((t, 2), (t, 2))
        doubles = [(e, [0, 1]) for e in ex] + [(e, [2, 3]) for e in ex]
        cores = [[doubles[2 * c], doubles[2 * c + 1]] for c in range(8)]
        return sets, cores
    if na == 6:
        b = ts[0]
        sets = ((a, 1), (a, 2), (b, 1))
        doubles = ([(A[0], [0, 1]), (A[0], [2, 3]),
                    (A[1], [0, 1]), (A[1], [2, 3])]
                   + [(e, [0, 1]) for e in A[2:6]])
        singles = [(e, [c]) for e in A[2:6] for c in (2, 3)]
        bsing = [(e, [c]) for e in B for c in range(NCH)]
        cores = [[singles[c], doubles[c], bsing[c]] for c in range(8)]
        return sets, cores
    if na == 4:
        b = ts[0]
        sets = ((a, 1), (a, 1), (b, 2))
        singles = [(e, [c]) for e in A for c in range(NCH)]        # 16
        bdoubles = [(e, [0, 1]) for e in B] + [(e, [2, 3]) for e in B]
        cores = [[singles[2 * c], singles[2 * c + 1], bdoubles[c]]
                 for c in range(8)]
        return sets, cores
    return None


def _plan_fallback(tcounts):
    """Expert-pair scheme (pads every set to the pair maxima)."""
    order = sorted(range(E), key=lambda e: -tcounts[e])
    pairs = [(order[i], order[E - 1 - i]) for i in range(E // 2)]
    T1 = max(tcounts[a] for a, _ in pairs)
    T2 = max(tcounts[b] for _, b in pairs)
    sets = ((T1, 2), (T2, 2))
    cores = []
    for a, b in pairs:
        for h in range(2):
            cores.append([(a, [2 * h, 2 * h + 1]), (b, [2 * h, 2 * h + 1])])
    return sets, cores


def _permute(arr):
    """[D, C] -> [P, KT*C]: per-partition-contiguous SBUF image."""
    C = arr.shape[1]
    return np.ascontiguousarray(
        arr.reshape(KT, P, C).transpose(1, 0, 2).reshape(P, KT * C))


def _halves(arr):
    """[D, C] -> (even, odd) [D/2, C] by 128-row d-chunk parity."""
    a = arr.reshape(KT, P, -1)
    return (np.ascontiguousarray(a[0::2].reshape(D // 2, -1)),
            np.ascontiguousarray(a[1::2].reshape(D // 2, -1)))


def run(inputs, trace=False, trace_cores=None):
    """Full pipeline. Returns (output, BassKernelResults)."""
    from concourse.bass_utils import run_bass_kernel_spmd

    x = np.asarray(inputs["x"], dtype=np.float32)
    W = np.asarray(inputs["W"], dtype=np.float32)
    V = np.asarray(inputs["V"], dtype=np.float32)
    router_w = np.asarray(inputs["router_w"])
    router_b = np.asarray(inputs["router_b"])
    k = int(np.asarray(inputs["top_k"]))

    B, T, d = x.shape
    assert d == D and W.shape == (E, D, H) and V.shape == (E, D, H)
    N = B * T
    xf = x.reshape(N, D)

    ids, wts = _route(xf, router_w, router_b, k)

    # Peel small ceil-remainders off to the host (exact fp32 there).
    host_jobs = []
    for e in range(E):
        n = len(ids[e])
        rem = n % P
        if 0 < rem <= REM_CAP and n > rem:
            host_jobs.append((e, ids[e][n - rem:], wts[e][n - rem:]))
            ids[e] = ids[e][:n - rem]
            wts[e] = wts[e][:n - rem]

    tcounts = [max(1, -(-len(i) // P)) for i in ids]     # tiles per expert

    plan = _plan(tcounts) or _plan_fallback(tcounts)
    sets, cores = plan
    nc = _get_kernel(sets)
    alias_ok = (len(sets) > 2
                and all(s <= sets[0][0] for s, _ in sets[2:])
                and sets[0][1] == 1)

    bf16 = ml_dtypes.bfloat16
    # Per-expert weight slabs [NCH, D, 512] in bf16, cast once.
    Wr = [np.ascontiguousarray(
        W[e].reshape(D, NCH, HSLAB).transpose(1, 0, 2)).astype(bf16)
        for e in range(E)]
    Vr = [np.ascontiguousarray(
        V[e].reshape(D, NCH, HSLAB).transpose(1, 0, 2)).astype(bf16)
        for e in range(E)]

    # Token-set tensors per (expert, padded tile count), built once.
    xts = {}

    def xt_for(e, tiles, perm):
        key = (e, tiles, perm)
        if key not in xts:
            n = len(ids[e])
            xt = np.zeros((D, tiles * P), dtype=bf16)
            xt[:, :n] = xf[ids[e]].T.astype(bf16)
            xts[key] = _permute(xt) if perm else xt
        return xts[key]

    in_maps = []
    jobs_meta = []  # per core: list of (expert, chunk, tiles)
    for c in range(N_CORES):
        assign = cores[c]
        e0, ch0 = assign[0][0], assign[0][1][0]
        xw0e, xw0o = _halves(np.concatenate(
            [xt_for(e0, sets[0][0], False), Wr[e0][ch0], Vr[e0][ch0]],
            axis=1))
        im = {"xw0e": xw0e, "xw0o": xw0o}
        meta = []
        rest = []
        for si, (e, chunks) in enumerate(assign):
            if si > 0:
                if si >= 2 and alias_ok:
                    xe, xo = _halves(xt_for(e, sets[si][0], False))
                    im[f"xt{si}e"] = xe
                    im[f"xt{si}o"] = xo
                else:
                    im[f"xt{si}"] = xt_for(e, sets[si][0], True)
            for j, ch in enumerate(chunks):
                if not (si == 0 and j == 0):
                    rest.append(np.stack(
                        [_permute(Wr[e][ch]), _permute(Vr[e][ch])]))
                meta.append((e, ch, sets[si][0]))
        if rest:
            im["wv"] = np.ascontiguousarray(np.stack(rest))
        in_maps.append(im)
        jobs_meta.append(meta)

    # The device pass very occasionally returns garbage (transient
    # runtime glitch).  The true output is mathematically bounded
    # (|y| < ~50 for this data), so NaN/Inf or absurd magnitudes
    # reliably detect a bad pass; retry once.
    for attempt in range(2):
        res = run_bass_kernel_spmd(
            nc,
            in_maps,
            core_ids=list(range(N_CORES)),
            trace=trace,
            trace_cores=trace_cores,
        )
        ok = True
        for r in res.results:
            y = r["out"].astype(np.float32)
            if not np.isfinite(y).all() or np.abs(y).max() > 1e3:
                ok = False
                break
        if ok:
            break

    outf = np.zeros((N, H), dtype=np.float32)
    for c in range(N_CORES):
        y = res.results[c]["out"].astype(np.float32)    # [U*P, 512]
        u = 0
        for e, ch, tiles in jobs_meta[c]:
            n = len(ids[e])
            for ct in range(tiles):
                lo = ct * P
                hi = min(lo + P, n)
                if hi > lo:
                    rows = ids[e][lo:hi]
                    outf[rows, ch * HSLAB:(ch + 1) * HSLAB] += (
                        wts[e][lo:hi, None] * y[(u + ct) * P:(u + ct) * P + (hi - lo)]
                    )
            u += tiles

    for e, toks, w in host_jobs:
        xe = xf[toks]                                   # [r, D] fp32
        a = xe @ W[e]
        b = xe @ V[e]
        outf[toks] += w[:, None] * (a * (b / (1.0 + np.exp(-b))))
    return outf.reshape(B, T, H), res


def kernel(**inputs):
    out, _ = run(inputs, trace=False)
    return out


# revision 49
# speedup vs baseline: 1.0960x; 1.0469x over previous
"""Trainium2 Bass kernel for nn_MixtureOfExperts (moe_routing).

Strategy (expert-parallel with exact unit balancing):
  - Host computes the tiny router (N x D @ D x E = 0.1% of total FLOPs),
    top-k selection and softmax combine weights in fp32 numpy.
  - Each expert's ceil-remainder tokens (n_e mod 128, <1 tile) are
    evaluated on the host in exact fp32 alongside the router, so the
    device computes only whole 128-token tiles.
  - Work is decomposed into units of (expert, 128-token tile, 512-col
    H-chunk); each unit is 2x16 accumulating bf16 matmuls (x@W and x@V
    slices) + SiLU + multiply + out-DMA.  With the graded routing the
    experts need 62 whole token-tiles -> 248 units, which split EXACTLY
    31 per core via per-core "slots": each core gets a few token-sets
    (an expert's routed tokens, transposed) and per set a number of
    weight chunk-slabs, e.g. (8 tiles x 1 slab) + (8 x 2) + (7 x 1).
    This removes the padding of the expert-pair scheme and hits the
    per-core PE floor (~212us issued matmul time).
  - DMA ring discipline (the tile scheduler hoists dependency-free
    DMA issues to the front of their ring, and a ring processes
    entries IN ORDER):
      * SP(sync) ring carries ONLY input loads, emitted in priority
        order: 16-chunk startup chase (set-0 tokens + first W,V slab,
        host-fused; chunk granularity feeds the PE ramp), then set-1
        tokens, then W/V slabs in job order.  Even if hoisted, the
        ring streams exactly in this order at full bandwidth.
      * Activation(scalar) ring carries ONLY output stores, which are
        data-dependent and pace themselves with compute.
  - Non-chase loads are host-PRE-PERMUTED to [128, KT*cols] so each
    DMA is 128 fat contiguous descriptors (16-37KB) instead of 2048
    1-2KB ones (the DGE is ~100ns/descriptor, so small descriptors
    waste engine time the chase needs).
  - Set 2's tokens alias set 0's SBUF region (dead after job 0), which
    frees room for a 5-deep W/V slab pool.
  - Outputs leave in bf16 (0.2% rel error, far inside the 2e-2 gate).
    Host scatter-adds the per-unit outputs weighted by the combine
    probabilities.
"""

import numpy as np
import ml_dtypes

P = 128
D = 2048
H = 2048
E = 8
N_CORES = 8
HSLAB = 512
NCH = H // HSLAB          # 4 chunk-slabs per expert
KT = D // P               # 16 contraction tiles
# An expert's ceil-remainder tokens (< P) cost a full 128-token tile on
# the PE.  Remainders are evaluated on the host in exact fp32 instead
# (alongside the replicated router), emptying the partial tile: the
# device computes only whole 128-token tiles.
REM_CAP = P - 1

_compiled = {}


def _build(sets):
    """Build the Bass/Tile kernel for per-core slots `sets`:
    a tuple of (tiles, nslabs) pairs.  Every core runs, for each set i,
    nslabs_i jobs of tiles_i token-tiles against one 512-col W/V slab
    each."""
    import concourse.bacc as bacc
    import concourse.mybir as mybir
    import concourse.tile as tile

    U = sum(s * k for s, k in sets)
    n_rest = sum(k for _, k in sets) - 1
    s0 = sets[0][0]
    C0 = s0 * P

    nc = bacc.Bacc("TRN2", target_bir_lowering=False, debug=False)
    bf16 = mybir.dt.bfloat16
    f32 = mybir.dt.float32

    # SBUF budget (KB/partition): x sets cost cols/32, wv slabs 16/buf,
    # elementwise sil 2/buf + ot(bf16) 1/buf.  Sets beyond the second
    # alias into set 0's x region (dead after job 0).
    alias_ok = (len(sets) > 2
                and all(s <= s0 for s, _ in sets[2:])
                and sets[0][1] == 1)
    x_cols = C0 + 2 * HSLAB + sum(s * P for s, _ in sets[1:2])
    if not alias_ok:
        x_cols += sum(s * P for s, _ in sets[2:])
    wv_bufs, elt_bufs = 5, 6
    while x_cols // 32 + 16 * wv_bufs + 3 * elt_bufs + 2 > 200 and wv_bufs > 2:
        wv_bufs -= 1

    XW0 = nc.dram_tensor("xw0", [D, C0 + 2 * HSLAB], bf16,
                         kind="ExternalInput").ap()
    xts_d = {}
    for i in range(1, len(sets)):
        if i >= 2 and alias_ok:
            # aliased sets keep the [D, C] layout (written through the
            # strided 3-D view of set 0's region)
            xts_d[i] = nc.dram_tensor(f"xt{i}", [D, sets[i][0] * P], bf16,
                                      kind="ExternalInput").ap()
        else:
            # pre-permuted: straight [P, KT*C] copy, 128 fat descriptors
            xts_d[i] = nc.dram_tensor(f"xt{i}", [P, KT * sets[i][0] * P],
                                      bf16, kind="ExternalInput").ap()
    WVr = (nc.dram_tensor("wv", [n_rest, 2, P, KT * HSLAB], bf16,
                          kind="ExternalInput").ap() if n_rest else None)
    out = nc.dram_tensor("out", [U * P, HSLAB], bf16,
                         kind="ExternalOutput").ap()

    # job list: (set index, slab index within set)
    jobs = [(si, sj) for si, (s, k) in enumerate(sets) for sj in range(k)]
    first_job = {}
    for j, (si, _) in enumerate(jobs):
        first_job.setdefault(si, j)
    # ring position for set i's x load: right after the slab loads of
    # job first_job(i)-2 (two jobs of lead time; clamps to pre-loop)
    defer = {}
    for si in range(1, len(sets)):
        defer.setdefault(max(0, first_job[si] - 2), []).append(si)

    with tile.TileContext(nc) as tc:
        with (
            tc.tile_pool(name="xt", bufs=1) as xt_pool,
            tc.tile_pool(name="wv", bufs=wv_bufs) as wv_pool,
            tc.tile_pool(name="elt", bufs=elt_bufs) as elt_pool,
            tc.tile_pool(name="psum", bufs=8, space="PSUM") as psum_pool,
        ):
            xw0_sb = xt_pool.tile([P, KT, C0 + 2 * HSLAB], bf16, tag="xw0")
            # ("c", 3-D chunked view) or ("f", flat)
            xt_sbs = {0: ("c", xw0_sb[:, :, :C0])}
            for i in range(1, len(sets)):
                if i >= 2 and alias_ok:
                    xt_sbs[i] = ("c", xw0_sb[:, :, :sets[i][0] * P])
                else:
                    t = xt_pool.tile([P, KT * sets[i][0] * P], bf16,
                                     tag=f"xt{i}", name=f"xt{i}_sb")
                    xt_sbs[i] = ("f", t)

            def lhs(si, d, ct):
                kind, t = xt_sbs[si]
                if kind == "c":
                    return t[:, d, ct * P:(ct + 1) * P]
                C = sets[si][0] * P
                return t[:, d * C + ct * P: d * C + (ct + 1) * P]

            def emit_xt_load(si):
                kind, t = xt_sbs[si]
                if kind == "c":
                    nc.sync.dma_start(
                        t, xts_d[si].rearrange("(ko p) n -> p ko n", p=P))
                else:
                    nc.sync.dma_start(t, xts_d[si])

            # HAM pre-warm: back-to-back N=128 matmuls alternating psum
            # banks keep the PE active during the DMA wait so the clock
            # gate is at 2.4GHz when the real stream starts.
            warm = xt_pool.tile([P, P], bf16, tag="warm")
            nc.gpsimd.memset(warm, 0.0)
            wps = [psum_pool.tile([P, HSLAB], f32, tag="ps", name=f"warm_ps{i}")
                   for i in range(2)]
            for i in range(32):
                nc.tensor.matmul(wps[i % 2][:, :P], warm, warm,
                                 start=True, stop=True)

            # startup chase on the input ring, chunk-granular
            for d in range(KT):
                nc.sync.dma_start(xw0_sb[:, d, :], XW0[d * P:(d + 1) * P, :])
            w00 = xw0_sb[:, :, C0:C0 + HSLAB]
            v00 = xw0_sb[:, :, C0 + HSLAB:]

            for si in defer.pop(0, []):
                emit_xt_load(si)

            unit = 0
            rest = 0
            for jidx, (si, sj) in enumerate(jobs):
                tiles = sets[si][0]
                if jidx == 0:

                    def rhs(mat, d, h0, h1):
                        return (w00 if mat == 0 else v00)[:, d, h0:h1]
                else:
                    wsl = wv_pool.tile([P, KT * HSLAB], bf16, tag="wv")
                    nc.sync.dma_start(wsl, WVr[rest, 0])
                    vsl = wv_pool.tile([P, KT * HSLAB], bf16, tag="wv")
                    nc.sync.dma_start(vsl, WVr[rest, 1])
                    rest += 1

                    def rhs(mat, d, h0, h1, _w=wsl, _v=vsl):
                        t = _w if mat == 0 else _v
                        return t[:, d * HSLAB + h0: d * HSLAB + h1]

                for si2 in defer.pop(jidx, []):
                    emit_xt_load(si2)

                def a_phase(ct):
                    pa = psum_pool.tile([P, HSLAB], f32, tag="ps")
                    for d in range(KT):
                        nc.tensor.matmul(
                            pa, lhs(si, d, ct), rhs(0, d, 0, HSLAB),
                            start=(d == 0), stop=(d == KT - 1),
                        )
                    return pa

                def b_phase_and_out(ct, pa, split=False):
                    u = unit + ct
                    # Final tile: half-width groups so SiLU/mul/DMA
                    # overlap the remaining matmuls (shortens the tail).
                    # 256-col matmuls (108ns) still hide the 97ns
                    # LDWEIGHTS; 128-col ones would expose it.
                    q = HSLAB // 2
                    halves = (
                        [(i * q, (i + 1) * q) for i in range(2)] if split
                        else [(0, HSLAB)]
                    )
                    for h0, h1 in halves:
                        nh = h1 - h0
                        pb = psum_pool.tile([P, HSLAB], f32, tag="ps")
                        for d in range(KT):
                            nc.tensor.matmul(
                                pb[:, :nh], lhs(si, d, ct), rhs(1, d, h0, h1),
                                start=(d == 0), stop=(d == KT - 1),
                            )
                        sil = elt_pool.tile([P, HSLAB], f32, tag="sil")
                        nc.scalar.activation(
                            sil[:, :nh], pb[:, :nh],
                            mybir.ActivationFunctionType.Silu,
                        )
                        ot = elt_pool.tile([P, HSLAB], bf16, tag="ot")
                        nc.vector.tensor_mul(ot[:, :nh], pa[:, h0:h1],
                                             sil[:, :nh])
                        nc.scalar.dma_start(
                            out[u * P:(u + 1) * P, h0:h1], ot[:, :nh]
                        )

                # First job: V slab races the PE up the ramp; lag the
                # b-phases a few tiles to give the chase arrival slack.
                lag = 3 if jidx == 0 else 0
                last_ct = tiles - 1 if jidx == len(jobs) - 1 else -1
                pending = []
                for ct in range(tiles):
                    pending.append((ct, a_phase(ct)))
                    if len(pending) > lag:
                        pct, ppa = pending.pop(0)
                        b_phase_and_out(pct, ppa, split=(pct == last_ct))
                for pct, ppa in pending:
                    b_phase_and_out(pct, ppa, split=(pct == last_ct))
                unit += tiles
    nc.compile()
    return nc


def _get_kernel(sets):
    key = tuple(sets)
    if key not in _compiled:
        _compiled[key] = _build(key)
    return _compiled[key]


def _route(xf, router_w, router_b, k):
    """fp32 router: per-expert token ids and softmax combine weights."""
    logits = xf @ router_w.astype(np.float32) + router_b.astype(np.float32)
    # stable: ties resolve to the lower expert index, like lax.top_k
    order = np.argsort(-logits, axis=1, kind="stable")[:, :k]   # [N, k]
    top_logits = np.take_along_axis(logits, order, axis=1)
    m = top_logits.max(axis=1, keepdims=True)
    p = np.exp(top_logits - m)
    p /= p.sum(axis=1, keepdims=True)                   # [N, k]
    ids, wts = [], []
    for e in range(E):
        mask = order == e                               # [N, k]
        tok = np.nonzero(mask.any(axis=1))[0]
        wt = (p * mask).sum(axis=1)[tok]
        ids.append(tok)
        wts.append(wt.astype(np.float32))
    return ids, wts


def _plan(tcounts):
    """Choose per-core slots.  Returns (sets, cores) where cores[c] is a
    list over sets of (expert, [chunk indices]); or None if the tile
    profile doesn't fit a balanced template."""
    ts = sorted(set(tcounts))
    if len(ts) == 1:
        a = ts[0]
        A, B = list(range(E)), []
    elif len(ts) == 2 and ts[1] - ts[0] == 1:
        b, a = ts
        A = [e for e in range(E) if tcounts[e] == a]
        B = [e for e in range(E) if tcounts[e] == b]
    else:
        return None
    na = len(A)
    if na % 2:
        return None
    if na in (0, 8):
        ex = A or B
        t = tcounts[ex[0]]
        sets = ((t, 2), (t, 2))
        doubles = [(e, [0, 1]) for e in ex] + [(e, [2, 3]) for e in ex]
        cores = [[doubles[2 * c], doubles[2 * c + 1]] for c in range(8)]
        return sets, cores
    if na == 6:
        b = ts[0]
        sets = ((a, 1), (a, 2), (b, 1))
        doubles = ([(A[0], [0, 1]), (A[0], [2, 3]),
                    (A[1], [0, 1]), (A[1], [2, 3])]
                   + [(e, [0, 1]) for e in A[2:6]])
        singles = [(e, [c]) for e in A[2:6] for c in (2, 3)]
        bsing = [(e, [c]) for e in B for c in range(NCH)]
        cores = [[singles[c], doubles[c], bsing[c]] for c in range(8)]
        return sets, cores
    if na == 4:
        b = ts[0]
        sets = ((a, 1), (a, 1), (b, 2))
        singles = [(e, [c]) for e in A for c in range(NCH)]        # 16
        bdoubles = [(e, [0, 1]) for e in B] + [(e, [2, 3]) for e in B]
        cores = [[singles[2 * c], singles[2 * c + 1], bdoubles[c]]
                 for c in range(8)]
        return sets, cores
    return None


def _plan_fallback(tcounts):
    """Expert-pair scheme (pads every set to the pair maxima)."""
    order = sorted(range(E), key=lambda e: -tcounts[e])
    pairs = [(order[i], order[E - 1 - i]) for i in range(E // 2)]
    T1 = max(tcounts[a] for a, _ in pairs)
    T2 = max(tcounts[b] for _, b in pairs)
    sets = ((T1, 2), (T2, 2))
    cores = []
    for a, b in pairs:
        for h in range(2):
            cores.append([(a, [2 * h, 2 * h + 1]), (b, [2 * h, 2 * h + 1])])
    return sets, cores


def _permute(arr):
    """[D, C] -> [P, KT*C]: per-partition-contiguous SBUF image."""
    C = arr.shape[1]
    return np.ascontiguousarray(
        arr.reshape(KT, P, C).transpose(1, 0, 2).reshape(P, KT * C))


def _halves(arr):
    """[D, C] -> (even, odd) [D/2, C] by 128-row d-chunk parity."""
    a = arr.reshape(KT, P, -1)
    return (np.ascontiguousarray(a[0::2].reshape(D // 2, -1)),
            np.ascontiguousarray(a[1::2].reshape(D // 2, -1)))


def run(inputs, trace=False, trace_cores=None):
    """Full pipeline. Returns (output, BassKernelResults)."""
    from concourse.bass_utils import run_bass_kernel_spmd

    x = np.asarray(inputs["x"], dtype=np.float32)
    W = np.asarray(inputs["W"], dtype=np.float32)
    V = np.asarray(inputs["V"], dtype=np.float32)
    router_w = np.asarray(inputs["router_w"])
    router_b = np.asarray(inputs["router_b"])
    k = int(np.asarray(inputs["top_k"]))

    B, T, d = x.shape
    assert d == D and W.shape == (E, D, H) and V.shape == (E, D, H)
    N = B * T
    xf = x.reshape(N, D)

    ids, wts = _route(xf, router_w, router_b, k)

    # Peel small ceil-remainders off to the host (exact fp32 there).
    host_jobs = []
    for e in range(E):
        n = len(ids[e])
        rem = n % P
        if 0 < rem <= REM_CAP and n > rem:
            host_jobs.append((e, ids[e][n - rem:], wts[e][n - rem:]))
            ids[e] = ids[e][:n - rem]
            wts[e] = wts[e][:n - rem]

    tcounts = [max(1, -(-len(i) // P)) for i in ids]     # tiles per expert

    plan = _plan(tcounts) or _plan_fallback(tcounts)
    sets, cores = plan
    nc = _get_kernel(sets)
    alias_ok = (len(sets) > 2
                and all(s <= sets[0][0] for s, _ in sets[2:])
                and sets[0][1] == 1)

    bf16 = ml_dtypes.bfloat16
    # Per-expert weight slabs [NCH, D, 512] in bf16, cast once.
    Wr = [np.ascontiguousarray(
        W[e].reshape(D, NCH, HSLAB).transpose(1, 0, 2)).astype(bf16)
        for e in range(E)]
    Vr = [np.ascontiguousarray(
        V[e].reshape(D, NCH, HSLAB).transpose(1, 0, 2)).astype(bf16)
        for e in range(E)]

    # Token-set tensors per (expert, padded tile count), built once.
    xts = {}

    def xt_for(e, tiles, perm):
        key = (e, tiles, perm)
        if key not in xts:
            n = len(ids[e])
            xt = np.zeros((D, tiles * P), dtype=bf16)
            xt[:, :n] = xf[ids[e]].T.astype(bf16)
            xts[key] = _permute(xt) if perm else xt
        return xts[key]

    in_maps = []
    jobs_meta = []  # per core: list of (expert, chunk, tiles)
    for c in range(N_CORES):
        assign = cores[c]
        e0, ch0 = assign[0][0], assign[0][1][0]
        xw0 = np.ascontiguousarray(np.concatenate(
            [xt_for(e0, sets[0][0], False), Wr[e0][ch0], Vr[e0][ch0]],
            axis=1))
        im = {"xw0": xw0}
        meta = []
        rest = []
        for si, (e, chunks) in enumerate(assign):
            if si > 0:
                perm = not (si >= 2 and alias_ok)
                im[f"xt{si}"] = xt_for(e, sets[si][0], perm)
            for j, ch in enumerate(chunks):
                if not (si == 0 and j == 0):
                    rest.append(np.stack(
                        [_permute(Wr[e][ch]), _permute(Vr[e][ch])]))
                meta.append((e, ch, sets[si][0]))
        if rest:
            im["wv"] = np.ascontiguousarray(np.stack(rest))
        in_maps.append(im)
        jobs_meta.append(meta)

    # The device pass very occasionally returns garbage (transient
    # runtime glitch).  The true output is mathematically bounded
    # (|y| < ~50 for this data), so NaN/Inf or absurd magnitudes
    # reliably detect a bad pass; retry once.
    for attempt in range(2):
        res = run_bass_kernel_spmd(
            nc,
            in_maps,
            core_ids=list(range(N_CORES)),
            trace=trace,
            trace_cores=trace_cores,
        )
        ok = True
        for r in res.results:
            y = r["out"].astype(np.float32)
            if not np.isfinite(y).all() or np.abs(y).max() > 1e3:
                ok = False
                break
        if ok:
            break

    outf = np.zeros((N, H), dtype=np.float32)
    for c in range(N_CORES):
        y = res.results[c]["out"].astype(np.float32)    # [U*P, 512]
        u = 0
        for e, ch, tiles in jobs_meta[c]:
            n = len(ids[e])
            for ct in range(tiles):
                lo = ct * P
                hi = min(lo + P, n)
                if hi > lo:
                    rows = ids[e][lo:hi]
                    outf[rows, ch * HSLAB:(ch + 1) * HSLAB] += (
                        wts[e][lo:hi, None] * y[(u + ct) * P:(u + ct) * P + (hi - lo)]
                    )
            u += tiles

    for e, toks, w in host_jobs:
        xe = xf[toks]                                   # [r, D] fp32
        a = xe @ W[e]
        b = xe @ V[e]
        outf[toks] += w[:, None] * (a * (b / (1.0 + np.exp(-b))))
    return outf.reshape(B, T, H), res


def kernel(**inputs):
    out, _ = run(inputs, trace=False)
    return out
